# revision 9
# baseline (speedup 1.0000x reference)
"""Trainium2 Bass kernel for nn_EosLayer (gated linear-attention recurrence).

Sharding: 8 cores = 4 batches x 2 sequence halves. Each core processes
T = 512 (warmup) + 2048 (output) timesteps of one batch. The warmup window
replaces cross-core state passing: the per-(k,d) decay o < 0.97 makes
history older than 512 steps contribute < 2e-7 relative.

Per-core layout is d-major (d on partitions, time on the free dim):
  - host pre-transposes x to (d, t), so all GEMMs (i/e/s projections and
    the final W_out) contract over d on the partition axis with no
    on-device transposes
  - the recurrence m_t = o*m + e_t*i_t runs on the hardware prefix-scan
    (tensor_tensor_scan) with 4096 independent (k,d) lanes, chained across
    chunks via the `initial` operand
  - LayerNorm stats are partition reductions -> ones-vector matmuls;
    gamma/beta are folded into W_out on the host.
"""

import numpy as np

D = 512
K = 8
TAU = 16.0
EPS = 1e-5
B = 4
N = 4096
H = N // 2          # rows per core (output)
W = 512             # warmup rows
T = W + H           # 2560 rows processed per core
TC = 512            # chunk length (free-dim columns per chunk)
NCHUNK = T // TC    # 5 chunks; chunk 0 is pure warmup
NDO = 4             # d-tiles of 128 partitions
P = 128

_CACHE = {}


def _build():
    import concourse.bass as bass
    import concourse.mybir as mybir
    import concourse.tile as tile
    from concourse.bacc import Bacc

    f32 = mybir.dt.float32
    f32r = mybir.dt.float32r
    AF = mybir.ActivationFunctionType
    OP = mybir.AluOpType

    nc = Bacc("TRN2", target_bir_lowering=False, debug=False,
              enable_asserts=False, num_devices=8)

    # per-core input (pre-transposed x slice), shared weight/const inputs
    xt = nc.dram_tensor("xt", (D, T), f32r, kind="ExternalInput")
    wi = nc.dram_tensor("wi", (D, D), f32r, kind="ExternalInput")
    wes = nc.dram_tensor("wes", (D, 2 * K), f32r, kind="ExternalInput")
    oc = nc.dram_tensor("oc", (D, K), f32, kind="ExternalInput")      # o.T
    wo = nc.dram_tensor("wo", (D, D), f32r, kind="ExternalInput")     # gamma-folded
    hrow = nc.dram_tensor("hrow", (1, D), f32r, kind="ExternalInput")  # colsum(wo)
    bowr = nc.dram_tensor("bowr", (1, D), f32, kind="ExternalInput")   # beta @ W_out
    yout = nc.dram_tensor("yout", (H, D), f32, kind="ExternalOutput")

    with tile.TileContext(nc) as tc:
        with tc.tile_pool(name="const", bufs=1) as cst, \
             tc.tile_pool(name="state", bufs=1) as stp, \
             tc.tile_pool(name="work", bufs=2) as wk, \
             tc.tile_pool(name="big", bufs=1) as big, \
             tc.tile_pool(name="pmm", bufs=4, space="PSUM") as pmm, \
             tc.tile_pool(name="pes", bufs=1, space="PSUM") as pes, \
             tc.tile_pool(name="pg", bufs=1, space="PSUM") as pg, \
             tc.tile_pool(name="dr", bufs=2, space="DRAM") as dr:

            # ---- constants (loaded once) ----
            wi_sb = [cst.tile([P, D], f32r, tag=f"wi{t}", name=f"wi{t}") for t in range(NDO)]
            wes_sb = [cst.tile([P, 2 * K], f32r, tag=f"wes{t}", name=f"wes{t}") for t in range(NDO)]
            oc_sb = [cst.tile([P, K], f32, tag=f"oc{t}", name=f"oc{t}") for t in range(NDO)]
            wo_sb = [cst.tile([P, D], f32r, tag=f"wo{t}", name=f"wo{t}") for t in range(NDO)]
            for t in range(NDO):
                sl = slice(t * P, (t + 1) * P)
                nc.sync.dma_start(out=wi_sb[t], in_=wi[sl, :])
                nc.sync.dma_start(out=wes_sb[t], in_=wes[sl, :])
                nc.sync.dma_start(out=oc_sb[t], in_=oc[sl, :])
                nc.sync.dma_start(out=wo_sb[t], in_=wo[sl, :])
            h_sb = cst.tile([1, D], f32r, tag="h", name="h")
            nc.sync.dma_start(out=h_sb, in_=hrow[:, :])
            bow_rep = cst.tile([P, D], f32, tag="bow", name="bow")
            bsrc = bass.AP(tensor=bowr, offset=0, ap=[[0, P], [1, D]])
            nc.sync.dma_start(out=bow_rep, in_=bsrc)
            ones_sb = cst.tile([P, 1], f32r, tag="ones", name="ones")
            nc.vector.memset(ones_sb.bitcast(f32), 1.0)
            eps_sb = cst.tile([P, 1], f32, tag="eps", name="eps")
            nc.vector.memset(eps_sb, EPS)

            # ---- persistent state: scan carries ----
            carry = [stp.tile([P, K], f32, tag=f"carry{t}", name=f"carry{t}") for t in range(NDO)]

            for c in range(NCHUNK):
                is_warm = (c == 0)
                csl = slice(c * TC, (c + 1) * TC)

                # 1. load x^T chunk
                xt_sb = [wk.tile([P, TC], f32r, tag=f"xt{t}", name=f"xt{t}") for t in range(NDO)]
                for t in range(NDO):
                    nc.sync.dma_start(out=xt_sb[t],
                                      in_=xt[t * P:(t + 1) * P, csl])

                # 2. projections  iT = W_i^T x^T,  esT = [W_e|W_s]^T x^T
                it_ps = [pmm.tile([P, TC], f32, tag="itps", name="itps") for _ in range(NDO)]
                for m in range(NDO):
                    for kt in range(NDO):
                        nc.tensor.matmul(
                            it_ps[m][:, :],
                            wi_sb[kt][:, m * P:(m + 1) * P],
                            xt_sb[kt][:, :],
                            start=(kt == 0), stop=(kt == NDO - 1))
                es_ps = pes.tile([2 * K, TC], f32, tag="esps", name="esps")
                for kt in range(NDO):
                    nc.tensor.matmul(es_ps[:, :], wes_sb[kt][:, :],
                                     xt_sb[kt][:, :],
                                     start=(kt == 0), stop=(kt == NDO - 1))

                # 3. evacuate psum -> sbuf (scalar engine)
                it_sb = [wk.tile([P, TC], f32, tag=f"it{t}", name=f"it{t}", bufs=1) for t in range(NDO)]
                for t in range(NDO):
                    nc.scalar.copy(out=it_sb[t][:, :], in_=it_ps[t][:, :])
                es_sb = wk.tile([2 * K, TC], f32, tag="es", name="es")
                nc.scalar.copy(out=es_sb[:, :], in_=es_ps[:, :])

                # 4. replicate e (and s) rows across partitions: SBUF
                # sources cannot partition-broadcast, so bounce through DRAM
                es_d = dr.tile([2 * K, TC], f32, tag="esd", name="esd")
                nc.sync.dma_start(out=es_d[:, :], in_=es_sb[:, :])
                e_rep = big.tile([P, K * TC], f32, tag="esrep", name="erep",
                                 bufs=2)
                esrc = bass.AP(tensor=es_d.tensor, offset=es_d.offset,
                               ap=[[0, P], [TC, K], [1, TC]])
                nc.sync.dma_start(out=e_rep[:, :], in_=esrc)
                if not is_warm:
                    s_rep = big.tile([P, K * TC], f32, tag="esrep", name="srep",
                                     bufs=2)
                    ssrc = bass.AP(tensor=es_d.tensor,
                                   offset=es_d.offset + K * TC,
                                   ap=[[0, P], [TC, K], [1, TC]])
                    nc.sync.dma_start(out=s_rep[:, :], in_=ssrc)

                # 5-7. per d-tile: z = e*i, scan, y = sum_k s*m
                yt_sb = []
                y2_sb = []
                for t in range(NDO):
                    eng = nc.vector if t < 2 else nc.gpsimd
                    zm = big.tile([P, K * TC], f32, tag=f"zm{t}", name=f"zm{t}")
                    # z[d,(k,t)] = iT[d,t] * e_rep[d,(k,t)]
                    it3 = bass.AP(tensor=it_sb[t].tensor, offset=it_sb[t].offset,
                                  ap=[it_sb[t].ap[0], [0, K], [1, TC]])
                    er3 = e_rep[:, :].rearrange("p (k t) -> p k t", k=K)
                    zm3 = zm[:, :].rearrange("p (k t) -> p k t", k=K)
                    eng.tensor_mul(out=zm3, in0=er3, in1=it3)
                    # scan per k lane-group (in place: m overwrites z)
                    for k in range(K):
                        col = oc_sb[t][:, k:k + 1]
                        dec = bass.AP(tensor=col.tensor, offset=col.offset,
                                      ap=[col.ap[0], [0, TC]])
                        init = 0.0 if c == 0 else carry[t][:, k:k + 1]
                        nc.vector.tensor_tensor_scan(
                            out=zm[:, k * TC:(k + 1) * TC],
                            data0=dec,
                            data1=zm[:, k * TC:(k + 1) * TC],
                            initial=init,
                            op0=OP.mult, op1=OP.add)
                    # save carries (last column of each k block)
                    nc.scalar.copy(
                        out=carry[t][:, :],
                        in_=bass.AP(tensor=zm.tensor, offset=zm.offset + TC - 1,
                                    ap=[zm.ap[0], [TC, K]]))
                    if is_warm:
                        continue
                    # y = sum_k s*m : multiply then tree-reduce over k
                    sr3 = s_rep[:, :].rearrange("p (k t) -> p k t", k=K)
                    eng.tensor_mul(out=zm3, in0=zm3, in1=sr3)
                    half = K * TC // 2
                    eng.tensor_add(out=zm[:, 0:half], in0=zm[:, 0:half],
                                   in1=zm[:, half:2 * half])
                    q = half // 2
                    eng.tensor_add(out=zm[:, 0:q], in0=zm[:, 0:q],
                                   in1=zm[:, q:2 * q])
                    yt = wk.tile([P, TC], f32r, tag=f"yt{t}", name=f"yt{t}", bufs=1)
                    eng.tensor_add(out=yt[:, :], in0=zm[:, 0:TC],
                                   in1=zm[:, TC:2 * TC])
                    # y^2 for variance (scalar engine)
                    y2 = wk.tile([P, TC], f32r, tag=f"y2{t}", name=f"y2{t}", bufs=1)
                    nc.scalar.activation(out=y2[:, :], in_=yt[:, :],
                                         func=AF.Square, scale=1.0)
                    yt_sb.append(yt)
                    y2_sb.append(y2)

                if is_warm:
                    continue

                # 8. LN stats via ones-matmuls: M = sum_d y, Q = sum_d y^2
                m_ps = pes.tile([1, TC], f32, tag="mps", name="mps")
                q_ps = pes.tile([1, TC], f32, tag="qps", name="qps")
                for t in range(NDO):
                    nc.tensor.matmul(m_ps[:, :], ones_sb[:, :], yt_sb[t][:, :],
                                     start=(t == 0), stop=(t == NDO - 1))
                for t in range(NDO):
                    nc.tensor.matmul(q_ps[:, :], ones_sb[:, :], y2_sb[t][:, :],
                                     start=(t == 0), stop=(t == NDO - 1))
                m_sb = wk.tile([1, TC], f32, tag="msb", name="msb")
                q_sb = wk.tile([1, TC], f32, tag="qsb", name="qsb")
                nc.scalar.copy(out=m_sb[:, :], in_=m_ps[:, :])
                nc.scalar.copy(out=q_sb[:, :], in_=q_ps[:, :])
                # row of -mu = M * (-1/512) for the rank-1 G correction
                mneg = wk.tile([1, TC], f32r, tag="mneg", name="mneg")
                nc.vector.tensor_scalar_mul(out=mneg[:, :], in0=m_sb[:, :],
                                            scalar1=-1.0 / D)
                # rsig row = 1/sqrt(Q/D - (M/D)^2 + eps), on (1,TC) rows
                ntt = TC // P
                mu = wk.tile([1, TC], f32, tag="mu", name="mu")
                nc.vector.tensor_scalar_mul(out=mu[:, :], in0=m_sb[:, :],
                                            scalar1=1.0 / D)
                mu2 = wk.tile([1, TC], f32, tag="mu2", name="mu2")
                nc.vector.tensor_mul(out=mu2[:, :], in0=mu[:, :], in1=mu[:, :])
                var = wk.tile([1, TC], f32, tag="var", name="var")
                nc.vector.scalar_tensor_tensor(out=var[:, :], in0=q_sb[:, :],
                                               scalar=1.0 / D, in1=mu2[:, :],
                                               op0=OP.mult, op1=OP.subtract)
                sig = wk.tile([1, TC], f32, tag="sig", name="sig")
                nc.scalar.activation(out=sig[:, :], in_=var[:, :],
                                     func=AF.Sqrt, bias=eps_sb[0:1, :],
                                     scale=1.0)
                rsigrow = wk.tile([1, TC], f32, tag="rsigrow", name="rsigrow")
                nc.vector.reciprocal(out=rsigrow[:, :], in_=sig[:, :])
                # transpose rsig row slices into columns for the epilogue
                rsig = wk.tile([P, ntt], f32, tag="rsig", name="rsig")
                for tt in range(ntt):
                    r = rsigrow[0:1, tt * P:(tt + 1) * P]
                    nc.sync.dma_start(out=rsig[:, tt:tt + 1], in_=r)

                # 9. G = y^T @ Wo' (+ (-mu) x h), epilogue, store
                for tt in range(ntt):
                    g_ps = pg.tile([P, D], f32, tag="gps", name="gps")
                    tsl = slice(tt * P, (tt + 1) * P)
                    for t in range(NDO):
                        nc.tensor.matmul(g_ps[:, :], yt_sb[t][:, tsl],
                                         wo_sb[t][:, :],
                                         start=(t == 0), stop=False)
                    nc.tensor.matmul(g_ps[:, :], mneg[:, tsl], h_sb[:, :],
                                     start=False, stop=True)
                    out_sb = wk.tile([P, D], f32, tag=f"out{tt % 2}", name=f"out{tt % 2}")
                    nc.vector.scalar_tensor_tensor(
                        out=out_sb[:, :], in0=g_ps[:, :],
                        scalar=rsig[:, tt:tt + 1], in1=bow_rep[:, :],
                        op0=OP.mult, op1=OP.add)
                    orow = (c - 1) * TC + tt * P
                    nc.sync.dma_start(out=yout[orow:orow + P, :],
                                      in_=out_sb[:, :])

    nc.compile()
    return nc


def _prep_inputs(x, W_i, W_e, W_s, o_param, ln_gamma, ln_beta, W_out):
    o = np.exp(np.log1p(np.exp(-np.abs(o_param))) * (-1.0 / TAU)
               + np.minimum(o_param, 0.0) / TAU).astype(np.float32)
    # stable logsigmoid: log sigmoid(w) = min(w,0) - log1p(exp(-|w|))
    wes = np.concatenate([W_e, W_s], axis=1).astype(np.float32)
    wo = (ln_gamma[:, None] * W_out).astype(np.float32)
    hrow = wo.sum(axis=0, keepdims=True).astype(np.float32)
    bowr = (ln_beta @ W_out).astype(np.float32)[None, :]
    shared = {
        "wi": np.ascontiguousarray(W_i, np.float32),
        "wes": np.ascontiguousarray(wes),
        "oc": np.ascontiguousarray(o.T),
        "wo": np.ascontiguousarray(wo),
        "hrow": np.ascontiguousarray(hrow),
        "bowr": np.ascontiguousarray(bowr),
    }
    in_maps = []
    for core in range(8):
        b, h = core // 2, core % 2
        t0 = h * H
        lo = t0 - W
        if lo < 0:
            xs = np.concatenate(
                [np.zeros((W, D), np.float32), x[b, 0:t0 + H]], axis=0)
        else:
            xs = x[b, lo:t0 + H]
        m = dict(shared)
        m["xt"] = np.ascontiguousarray(xs.T, np.float32)
        in_maps.append(m)
    return in_maps


def kernel(x, W_i, W_e, W_s, o_param, ln_gamma, ln_beta, W_out):
    from concourse.bass_utils import run_bass_kernel_spmd

    if "nc" not in _CACHE:
        _CACHE["nc"] = _build()
    nc = _CACHE["nc"]

    in_maps = _prep_inputs(np.asarray(x, np.float32), np.asarray(W_i),
                           np.asarray(W_e), np.asarray(W_s),
                           np.asarray(o_param), np.asarray(ln_gamma),
                           np.asarray(ln_beta), np.asarray(W_out))
    res = run_bass_kernel_spmd(nc, in_maps, core_ids=list(range(8)))
    out = np.empty((B, N, D), np.float32)
    for core in range(8):
        b, h = core // 2, core % 2
        out[b, h * H:(h + 1) * H] = res.results[core]["yout"]
    return out
